# revision 28
# baseline (speedup 1.0000x reference)
"""EnergyAttention Trainium2 kernel (8-core SPMD, head/q hybrid sharding), v2.

reference math:
    K = einsum('kd,hzd->khz', g, Wk); Q = einsum('qd,hzd->qhz', g, Wq)
    scores = beta * einsum('qhz,khz->hqk', Q, K)        # [H, N, N]
    A = logsumexp(scores, -1); out = (-1/beta) * A.sum()

Sharding (no collectives; final scalar reduction on host):
    core c owns head A = c (all 2048 q rows) and head B = 8 + c//2
    restricted to q rows [1024*(c%2), 1024*(c%2)+1024).  Identical SPMD
    program on every core; the B-head q-half is selected by feeding g with
    its halves swapped on the qlo=0 cores (LSE is invariant to q/k
    permutations since we only ever sum over rows and reduce over k).

v2 design (vs the 104.7us baseline, which was engine-ping-pong bound):
  - g and W are re-laid-out and quantized on the host: gt/wt arrive as
    fp8e4 (x32 per operand) in exactly the [d-pair-packed] layout the PE
    wants, so there are no on-device transposes at all and the input DMA
    is half the bytes (1.6MB).
  - projections K^T,Q^T ([z 128][n]) run as fp8 DoubleRow matmuls
    (contraction 256/instr): 2x fewer PE cycles than bf16.  Both heads'
    z-dims are packed into the 128 output partitions, which makes the
    half-core B-head Q projection free (shared stream).
  - scores stay bf16 (contraction is 64, so the PE is output-column
    bound and fp8 wouldn't help); 24 units of [128 q, 2048 k] per core,
    4 matmuls each into one 4-bank PSUM tile (pool of 2).
  - the fp8/beta descaling is folded into the projection copies (which
    run on ACT, hidden in the DMA prefix before the first exp), so the
    PSUM scores ARE beta*QK and the stats need no rescaling.
  - the max pass doubles as the PSUM evacuation: one standard
    tensor_scalar per unit (out = scores * -1 -> bf16 SBUF ring,
    accum_out with op1=min -> -max).  A DVE op runs at 1 elem/lane/cycle
    from PSUM no matter what, so the copy rides the mandatory max scan
    for free, and the PSUM ring is freed by the DVE, not the ACT: the
    2-slot PSUM ring only spans PE->DVE ((PE+DVE)/2 < DVE), and the
    3-deep SBUF ring decouples DVE->ACT.  DVE is the steady-state wall
    at ~2.3us/unit.  (The fancier fused custom DVE ops -
    tensor_tensor_reduce / tensor_mask_reduce - die at runtime here:
    the ant-dve uop tables are never loaded.  tensor_scalar is a
    standard sundagen op and works.)
  - one 2048-wide ACT exp per unit from the SBUF ring (scale=-1
    un-negates the staged copy, bias=-max) with fused row-sum
    accumulator; stats land in one SBUF tile, single output DMA at the
    end.
  - a dummy exp at t=0 pulls the ~2.7us ACT table load into the DMA
    prefix.
  - LSE = m + log(l) is exact for any m (self-correcting), so all the
    scaling/rounding of the max path only has to keep exp in range.
"""

import numpy as np
import ml_dtypes
from contextlib import ExitStack

import concourse.bass as bass
import concourse.mybir as mybir
import concourse.tile as tile
from concourse import bacc
from concourse.bass_utils import run_bass_kernel_spmd

N, D, H, Y = 2048, 768, 12, 64
NCORES = 8
BETA = 1.0 / 8.0
DT = mybir.dt.float32
DTB = mybir.dt.bfloat16
DT8 = mybir.dt.float8e4

FP8_SCALE = 32.0          # per-operand fp8 scale for g and W
# psum projections = 1024*K (resp 1024*Q); the copies to bf16 descale and
# fold beta into K, so the score matmuls produce beta*Q.K = s_true exactly
KT_SCALE = 1.0 / (8.0 * 1024.0)
QT_SCALE = 1.0 / 1024.0

NU = 26                   # stat jobs: 22 full units + 4 half-units


def _build_kernel():
    nc = bacc.Bacc("TRN2", target_bir_lowering=False, debug=False, num_devices=1)
    g8_ap = nc.dram_tensor("g8", [128, 6 * N], DT8, kind="ExternalInput").ap()
    wq_ap = nc.dram_tensor("wq8", [128, 768], DT8, kind="ExternalInput").ap()
    wk_ap = nc.dram_tensor("wk8", [128, 768], DT8, kind="ExternalInput").ap()
    out_ap = nc.dram_tensor("stats", [128, 2 * NU], DT, kind="ExternalOutput").ap()

    AF = mybir.ActivationFunctionType
    OP = mybir.AluOpType
    DR = mybir.MatmulPerfMode.DoubleRow

    with tile.TileContext(nc) as tc, ExitStack() as ctx:
        sb = ctx.enter_context(tc.tile_pool(name="sb", bufs=1))
        warm = sb.tile([128, 1], DT)
        nc.gpsimd.memset(warm[:], 0.0)
        # pulls the exp table load into the DMA prefix
        nc.scalar.activation(warm[:], warm[:], AF.Exp)

        # w[p, t2, sub, z] = 32*W[z, 128*(2*t2+sub)+p] (beta NOT folded)
        # gt[p, c, t, i] = 32*g[512c+i, 128t+p]; each n-chunk c is a
        # contiguous 3KB per partition, so its DMA is one descriptor/row.
        # DMA order = first-use order: wk + gt c0 gate the kt stream.
        wq_sb = sb.tile([128, 3, 2, 128], DT8)
        wk_sb = sb.tile([128, 3, 2, 128], DT8)
        gt = sb.tile([128, 4, 6, 512], DT8)
        g8_r = g8_ap.rearrange("p (c t i) -> p c t i", c=4, t=6)
        # two HWDGE queues in parallel: sync carries gt c0 (split so the
        # first d-pair lands ASAP) and c2; the ACT queue (idle during the
        # prefix) carries the W's, c1 and c3
        nc.sync.dma_start(gt[:, 0, 0:2], g8_r[:, 0, 0:2])
        nc.scalar.dma_start(wk_sb[:], wk_ap.rearrange("p (a b z) -> p a b z", a=3, b=2))
        nc.sync.dma_start(gt[:, 0, 2:6], g8_r[:, 0, 2:6])
        nc.scalar.dma_start(wq_sb[:], wq_ap.rearrange("p (a b z) -> p a b z", a=3, b=2))
        nc.scalar.dma_start(gt[:, 1], g8_r[:, 1])
        nc.sync.dma_start(gt[:, 2], g8_r[:, 2])
        nc.scalar.dma_start(gt[:, 3], g8_r[:, 3])

        kt_sb = sb.tile([128, N], DTB)   # rows 0:64 = head A z, 64:128 = head B z
        qt_sb = sb.tile([128, N], DTB)
        stats = sb.tile([128, 2 * NU], DT)

        pp = ctx.enter_context(tc.tile_pool(name="pp", bufs=2, space="PSUM"))
        # SBUF staging ring for negated bf16 scores (DVE -> ACT edge)
        sc_pool = ctx.enter_context(tc.tile_pool(name="sc", bufs=3))

        # dummy matmuls while the input DMAs are in flight: the PE p-state
        # ramps to 2.4GHz only after ~3us of continuous execution, so burn
        # the DMA wait warming it up instead of starting the projections
        # at 1.2GHz.  Results land in a PSUM slot nobody reads.
        dumm = sb.tile([128, 512], DTB)
        nc.gpsimd.memset(dumm[:], 0.0)
        wt_ps = pp.tile([128, 2048], DT, tag="u", name="pewarm")
        for i in range(8):
            nc.tensor.matmul(
                wt_ps[0:64, 0:512], lhsT=dumm[:, 0:64], rhs=dumm[:],
                start=True, stop=True,
            )

        # ---- projections: per 512-col n-chunk c, 3+3 DoubleRow matmuls
        # (contraction 2x128 per instr) into one [128,512] bank each, then
        # descaling ACT copies to bf16 SBUF (ACT is idle during the prefix).
        # kt for all 4 n-chunks first (score units need the full k range),
        # then qt (only qt chunk 0 gates the first unit; the qt tail
        # overlaps the first score units).  One [128,2048] PSUM tile per
        # projection, one bank per n-chunk, 3 DoubleRow matmuls
        # (contraction 256) each; descaling ACT copies follow per bank.
        for w_sb, dst_sb, scale in (
            (wk_sb, kt_sb, KT_SCALE),
            (wq_sb, qt_sb, QT_SCALE),
        ):
            pt = pp.tile([128, 2048], DT, tag="u", name="proj")
            for c in range(4):
                for t2 in range(3):
                    nc.tensor.matmul(
                        pt[:, 512 * c : 512 * (c + 1)],
                        lhsT=w_sb[:, t2],
                        rhs=gt[:, c, 2 * t2 : 2 * t2 + 2, :],
                        start=(t2 == 0),
                        stop=(t2 == 2),
                        perf_mode=DR,
                    )
                nc.scalar.mul(
                    dst_sb[:, 512 * c : 512 * (c + 1)],
                    pt[:, 512 * c : 512 * (c + 1)],
                    scale,
                )

        # ---- score jobs: (head, q-block, k-range).  B-head units use
        # fixed q-blocks 8..15; host swaps g halves on qlo=0 cores.  The
        # last two units are split into k-halves (host merges their LSEs)
        # to halve the end-of-kernel pipeline drain.
        # stats layout per job i: col 2i = -m, col 2i+1 = l
        jobs = (
            [(0, j, 0, 2048) for j in range(16)]
            + [(1, 8 + j, 0, 2048) for j in range(6)]
            + [(1, 14, 0, 1024), (1, 14, 1024, 2048),
               (1, 15, 0, 1024), (1, 15, 1024, 2048)]
        )
        for u, (hb, j, klo, khi) in enumerate(jobs):
            kw = khi - klo
            r0 = 64 * hb
            lhsT = qt_sb[r0 : r0 + 64, 128 * j : 128 * (j + 1)]
            negm = stats[:, 2 * u : 2 * u + 1]
            ut = pp.tile([128, 2048], DT, tag="u", name=f"u{u}")
            for subc in range(kw // 512):
                nc.tensor.matmul(
                    ut[:, 512 * subc : 512 * (subc + 1)],
                    lhsT=lhsT,
                    rhs=kt_sb[r0 : r0 + 64, klo + 512 * subc : klo + 512 * (subc + 1)],
                    start=True,
                    stop=True,
                )
            # fused evacuate+max: sc = -scores (bf16 SBUF), negm = -max;
            # frees the PSUM slot at the DVE instead of the ACT.
            sc = sc_pool.tile([128, 2048], DTB, tag="s", name=f"sc{u}")
            nc.vector.tensor_scalar(
                sc[:, 0:kw], ut[:, 0:kw], -1.0, None, OP.mult, OP.min,
                accum_out=negm,
            )
            # exp(scores - max): scale=-1 un-negates the staged copy
            nc.scalar.activation(
                sc[:, 0:kw], sc[:, 0:kw], AF.Exp, bias=negm, scale=-1.0,
                accum_out=stats[:, 2 * u + 1 : 2 * u + 2],
            )

            if u == 15:
                nc.sync.dma_start(out_ap[:, 0:32], stats[:, 0:32])
            elif u == 21:
                nc.sync.dma_start(out_ap[:, 32:44], stats[:, 32:44])
        nc.sync.dma_start(out_ap[:, 44 : 2 * NU], stats[:, 44 : 2 * NU])

    nc.compile()
    return nc


_NC_CACHE = {}


def _get_nc():
    if "nc" not in _NC_CACHE:
        _NC_CACHE["nc"] = _build_kernel()
    return _NC_CACHE["nc"]


def _relayout_w(w):
    # [64z per head A|B stacked, 768d] -> [128p, 3t2, 2sub, 128z] flattened,
    # with w8[p, t2, sub, z] = w[z, 128*(2*t2+sub)+p]
    return np.ascontiguousarray(
        w.T.reshape(3, 2, 128, 128).transpose(2, 0, 1, 3).reshape(128, 768)
    )


def _make_in_maps(np_inputs):
    fp8 = ml_dtypes.float8_e4m3
    g = np.asarray(np_inputs["g"], dtype=np.float32)
    Wq = np.asarray(np_inputs["Wq"], dtype=np.float32)
    Wk = np.asarray(np_inputs["Wk"], dtype=np.float32)

    g8 = np.clip(g * FP8_SCALE, -240.0, 240.0).astype(fp8)
    # gt[p, t, i] = g8[i, 128t+p]
    g8_sw = np.concatenate([g8[N // 2 :], g8[: N // 2]], axis=0)

    def g_layout(garr):
        # [p][c][t][i] with gt[p,c,t,i] = g[512c+i, 128t+p]
        return np.ascontiguousarray(
            garr.T.reshape(6, 128, 4, 512).transpose(1, 2, 0, 3).reshape(128, 6 * N)
        )

    gt_maps = [g_layout(g8_sw), g_layout(g8)]  # index by qlo half (c%2)

    in_maps = []
    for c in range(NCORES):
        hb = 8 + c // 2
        wq = np.clip(
            np.concatenate([Wq[c], Wq[hb]], axis=0) * FP8_SCALE, -240.0, 240.0
        ).astype(fp8)
        wk = np.clip(
            np.concatenate([Wk[c], Wk[hb]], axis=0) * FP8_SCALE, -240.0, 240.0
        ).astype(fp8)
        in_maps.append(
            {
                "g8": gt_maps[c % 2],
                "wq8": _relayout_w(wq),
                "wk8": _relayout_w(wk),
            }
        )
    return in_maps


def kernel(g, Wq, Wk):
    in_maps = _make_in_maps({"g": g, "Wq": Wq, "Wk": Wk})
    nc = _get_nc()
    res = run_bass_kernel_spmd(nc, in_maps, core_ids=list(range(NCORES)))

    total = 0.0
    for c in range(NCORES):
        stats = res.results[c]["stats"].astype(np.float64)  # [128, 52]
        negm = stats[:, 0:44:2]
        l = stats[:, 1:44:2]
        total += (np.log(l) - negm).sum()
        m0 = -stats[:, 44::4]
        l0 = stats[:, 45::4]
        m1 = -stats[:, 46::4]
        l1 = stats[:, 47::4]
        m = np.maximum(m0, m1)
        lh = l0 * np.exp(m0 - m) + l1 * np.exp(m1 - m)
        total += (m + np.log(lh)).sum()
    return np.float32(-(1.0 / BETA) * total)
